# revision 2
# baseline (speedup 1.0000x reference)
"""LocallyConnected2d (3x3, stride 1, no bias) Trainium2 Bass kernel.

Problem: x [16,32,64,64] f32, weight [1,32,32,62,62,9] f32
         out [16,32,62,62] f32 = per-output-pixel unshared 3x3 conv.

Strategy (8 NeuronCores, SPMD, no collectives):
  * Shard over OH: core c owns output rows oh in [8c, 8c+8) (OH=62 padded
    to 64 with zero weights; padded rows dropped on host).
  * Per core, one "group" per output column ow = 8 output rows x 16 batch.
    Contraction over (cin, kj) = 96 partitions; the 3 ki taps accumulate
    in PSUM. Per (group, half-of-rows, ki) one matmul:
        lhsT [96, (p2,o)=128]   = weights (stationary, contiguous cols)
        rhs  [96, (b,p)=128]    = x columns (moving)
        psum [(p2,o)=128, (b,p)=128]; p==half*4+p2 entries are outputs.
  * Extraction: 8 strided copies per 8-group batch, alternating VectorE /
    ScalarE (PSUM -> SBUF), then one ~1MB DMA out per core.
  * Host (numpy) pre-layouts all tensors so every DMA is contiguous:
    weights stream as 4 ow-tiles of ~4.7MB each, double buffered.

Memory per core: ~18.3MB weights + 3.9MB x (3 kj-shifted copies) + 1MB out
=> DMA-bound at roughly 55-70us; PE (~40us) hides under it.
"""

import numpy as np

import concourse.bass as bass
import concourse.mybir as mybir
import concourse.tile as tile
from concourse.bass_utils import run_bass_kernel_spmd

N_CORES = 8
B, C, O = 16, 32, 32
H = W = 64
OH = OW = 62
PH = 8          # output rows per core (8*8=64 >= 62)
HP = 10         # x rows needed per core (PH + 2)
F32 = mybir.dt.float32

OW_TILES = [(0, 16), (16, 16), (32, 16), (48, 14)]  # (ow0, tw) covering 62
WROW = 3 * 2 * 128                                  # weight elems per (ow): ki*half*(p2,o)


def _split_multi_waits(nc, maxw=1):
    """neuronxcc walrus in this container accepts only one sync-wait per
    instruction; hoist extras onto same-engine NOPs placed just before."""
    for f in nc.m.functions:
        for bb in f.blocks:
            new = []
            for inst in bb.instructions:
                si = inst.sync_info
                waits = list(si.on_wait) if si is not None and si.on_wait else []
                if len(waits) > maxw:
                    extras, keep = waits[:-maxw], waits[-maxw:]
                    for wt in extras:
                        new.append(
                            mybir.InstNoOp(
                                name=f"I-waitsplit-{nc.next_id()}",
                                opcode="NoOp",
                                engine=inst.engine,
                                sync_info=mybir.SyncInfo(on_wait=[wt], on_update=[]),
                            )
                        )
                    si.on_wait = keep
                new.append(inst)
            bb.instructions = new


def _build_nc():
    nc = bass.Bass("TRN2", target_bir_lowering=False, debug=False, num_devices=1)
    x3 = nc.dram_tensor("x3", [96, B * HP * W], F32, kind="ExternalInput").ap()
    wt = nc.dram_tensor("wt", [96, OW * WROW], F32, kind="ExternalInput").ap()
    out = nc.dram_tensor("out", [128, OW * 2 * B], F32, kind="ExternalOutput").ap()

    with tile.TileContext(nc) as tc:
        with (
            tc.tile_pool(name="xp", bufs=1) as xp,
            tc.tile_pool(name="wp", bufs=2) as wp,
            tc.tile_pool(name="pp", bufs=2, space="PSUM") as pp,
            tc.tile_pool(name="op", bufs=1) as op,
        ):
            x3s = xp.tile([96, B * HP * W], F32)
            nc.sync.dma_start(out=x3s[:, :], in_=x3[:, :])
            x3r = x3s.rearrange("q (b h w) -> q b h w", b=B, h=HP, w=W)

            outs = op.tile([128, OW * 2 * B], F32)
            osr = outs.rearrange("m (w h b) -> m w h b", h=2, b=B)

            for ow0, tw in OW_TILES:
                wts = wp.tile([96, tw * WROW], F32, tag="wt")
                nc.sync.dma_start(
                    out=wts[:, :], in_=wt[:, ow0 * WROW : (ow0 + tw) * WROW]
                )
                wtr = wts.rearrange("q (w k h m) -> q w k h m", w=tw, k=3, h=2, m=128)

                for b0, bs in [(0, 8), (8, tw - 8)]:
                    ps = pp.tile([128, 8 * 256], F32, tag="ps")
                    for g in range(bs):
                        owl = b0 + g
                        ow = ow0 + owl
                        for half in range(2):
                            lo = g * 256 + half * 128
                            for ki in range(3):
                                nc.tensor.matmul(
                                    ps[:, lo : lo + 128],
                                    wtr[:, owl, ki, half, :],
                                    x3r[:, :, ki : ki + PH, ow],
                                    start=(ki == 0),
                                    stop=(ki == 2),
                                )
                    psr = ps.rearrange(
                        "m (g h b p) -> m g h b p", g=8, h=2, b=B, p=PH
                    )
                    for pq in range(PH):
                        half, p2 = pq // 4, pq % 4
                        src = psr[32 * p2 : 32 * (p2 + 1), 0:bs, half, :, pq]
                        dst = osr[
                            32 * p2 : 32 * (p2 + 1), ow0 + b0 : ow0 + b0 + bs, half, :
                        ]
                        if pq % 2 == 0:
                            nc.vector.tensor_copy(dst, src)
                        else:
                            nc.scalar.copy(dst, src)

            nc.sync.dma_start(out=out[:, :], in_=outs[:, :])

    _split_multi_waits(nc)
    return nc


_NC_CACHE = []


def kernel(x, weight):
    x = np.asarray(x, dtype=np.float32)
    w6 = np.asarray(weight, dtype=np.float32)[0].reshape(O, C, OH, OW, 3, 3)

    # zero-pad: x rows 64->68 / cols 64->66, weight OH 62->64
    xp = np.pad(x, ((0, 0), (0, 0), (0, 4), (0, 2)))
    wpad = np.pad(w6, ((0, 0), (0, 0), (0, 2), (0, 0), (0, 0), (0, 0)))

    in_maps = []
    for core in range(N_CORES):
        oh0 = core * PH
        x3 = np.empty((C, 3, B, HP, W), np.float32)
        for kj in range(3):
            x3[:, kj] = xp[:, :, oh0 : oh0 + HP, kj : kj + W].transpose(1, 0, 2, 3)
        # wt[c, kj, ow, ki, half, p2, o] = w6[o, c, oh0+half*4+p2, ow, ki, kj]
        ws = wpad[:, :, oh0 : oh0 + PH].reshape(O, C, 2, 4, OW, 3, 3)
        wt = ws.transpose(1, 6, 4, 5, 2, 3, 0)
        in_maps.append(
            {
                "x3": np.ascontiguousarray(x3).reshape(96, -1),
                "wt": np.ascontiguousarray(wt).reshape(96, -1),
            }
        )

    if not _NC_CACHE:
        _NC_CACHE.append(_build_nc())
    nc = _NC_CACHE[0]

    res = run_bass_kernel_spmd(nc, in_maps, core_ids=list(range(N_CORES)))

    out = np.empty((B, O, N_CORES * PH, OW), np.float32)
    for core in range(N_CORES):
        # out_core[(p2,o), (ow, half, b)] -> out[b, o, oh0+half*4+p2, ow]
        oc = res.results[core]["out"].reshape(4, O, OW, 2, B)
        oc = oc.transpose(4, 1, 3, 0, 2).reshape(B, O, PH, OW)
        out[:, :, core * PH : (core + 1) * PH, :] = oc
    return out[:, :, :OH, :]
